# revision 15
# baseline (speedup 1.0000x reference)
"""GNN message-passing kernel for nn_GCN2_64630667870322 on 8 Trainium2 cores.

3-layer edge-message GNN:
    m_e   = relu(cat(h[src_e], efeat_e) @ Wm + bm)        per edge
    agg_v = sum_{e: dst_e == v} m_e                        segment sum
    h'_v  = relu(cat(h_v, agg_v) @ Wa + ba)                per node
Output: h3 squeezed -> [N, 32] f32.

Strategy (edge-parallel, dst-range sharded across the 8 cores):
  * Core k owns dst nodes [k*12500, (k+1)*12500); its edges are exactly
    those with dst in range, so per-destination partial sums stay local.
  * Message decomposition: m = relu(p[src] + q),
      p = h @ Wm[:Fin]            (node-level matmul, f32 table)
      q = cat(e, 1) @ [Wm[Fin:]; bm]   (edge matmul on PE, bias folded)
  * p[src]: dma_gather (custom SWDGE op) of 256B rows; node ids bucketed
    into 4 windows of <=32767 rows so indices fit int16.
  * Aggregation: dma_scatter_add into per-core HBM tables.  The HW op
    loses updates for duplicate indices WITHIN one instruction (verified
    on silicon), but separate instructions accumulate exactly.  Host-side
    bin packing therefore deals each destination's edges across bins of
    2048 positions such that every bin is duplicate-free; each bin is one
    scatter instruction.  Bins alternate between SWDGE queues 1/2 and two
    agg tables (summed during node apply) for descriptor-gen parallelism.
  * Between layers, per-core node states h'^T (feature-major, bf16) are
    AllGather'd; every core rebuilds the full f32 p-table locally.

kernel(**inputs) accepts FULL inputs, returns the FULL [100000, 32] f32
output; all sharding happens inside.
"""
import numpy as np
import ml_dtypes

BF16 = ml_dtypes.bfloat16


class Cfg:
    def __init__(self, n_nodes, bins_per_bucket, cg=4096, cs=2048, psg=8):
        self.NCORE = 8
        self.NBUCK = 4
        self.N = n_nodes
        assert n_nodes % self.NCORE == 0
        self.NSH = n_nodes // self.NCORE               # real nodes per core
        self.SH = ((self.NSH + 127) // 128 + 1) * 128  # padded rows (+trash)
        self.NTAB = self.NCORE * self.SH               # p-table rows
        assert self.NTAB % self.NBUCK == 0
        self.BUCK = self.NTAB // self.NBUCK            # gather window rows
        assert self.BUCK <= 32767 and self.SH <= 32767
        assert self.BUCK % self.SH == 0
        self.CG = cg                                   # gather/edge chunk
        self.CS = cs                                   # scatter bin size
        assert cg == 2 * cs and cs % 128 == 0
        self.K = bins_per_bucket                       # scatter bins/bucket
        assert (self.K * cs) % cg == 0
        self.CAP = self.K * cs                         # edges per (core,bucket)
        self.CPB = self.CAP // cg                      # chunks per bucket
        self.NCHUNK = self.NBUCK * self.CPB
        self.EPAD = self.NCHUNK * cg
        self.TRASH = self.SH - 1
        self.F = [64, 50, 25, 32]
        self.NG = cg // 128                            # edge groups / chunk
        self.PSG = psg                                 # groups per psum tile
        assert self.NG % self.PSG == 0
        self.NT = self.SH // 128                       # 128-row tiles / shard


FULL = Cfg(100000, bins_per_bucket=52)


# --------------------------------------------------------------------------
# Device program (SPMD — identical on all 8 cores)
# --------------------------------------------------------------------------

def build_nc(cfg, collectives=True, ablate=()):
    from concourse import bacc, mybir, tile
    from concourse.masks import make_identity

    dt = mybir.dt
    nc = bacc.Bacc(
        "TRN2",
        target_bir_lowering=False,
        debug=False,
    )
    F = cfg.F

    inp = {}

    def ext(name, shape, dtype):
        inp[name] = nc.dram_tensor(name, shape, dtype, kind="ExternalInput")
        return inp[name]

    p1tab = ext("p1tab", [cfg.NTAB, 64], dt.float32)
    eT = ext("eT", [65, cfg.EPAD], dt.bfloat16)
    gidx = ext("gidx", [cfg.NCHUNK, 128, cfg.CG // 16], dt.int16)
    sidx = ext("sidx", [cfg.NCHUNK, 128, cfg.CG // 16], dt.int16)
    hT1own = ext("hT1own", [64, cfg.SH], dt.bfloat16)
    for l in range(3):
        ext(f"R{l}", [65, F[l + 1]], dt.bfloat16)
        ext(f"WAh{l}", [F[l], F[l + 1]], dt.bfloat16)
        ext(f"WAa{l}", [F[l + 1], F[l + 1]], dt.bfloat16)
        ext(f"ba{l}", [F[l + 1], 1], dt.float32)
    for l in (1, 2):
        ext(f"PW{l}", [F[l], 64], dt.bfloat16)

    agg = [[nc.dram_tensor(f"agg{l}{ab}", [cfg.SH, 64], dt.float32)
            for ab in "ab"] for l in range(3)]
    ptab = [p1tab,
            nc.dram_tensor("p2tab", [cfg.NTAB, 64], dt.float32),
            nc.dram_tensor("p3tab", [cfg.NTAB, 64], dt.float32)]
    hsh = [hT1own,
           nc.dram_tensor("h2sh", [F[1], cfg.SH], dt.bfloat16),
           nc.dram_tensor("h3sh", [F[2], cfg.SH], dt.bfloat16)]
    hfull = [None,
             nc.dram_tensor("h2full", [cfg.NCORE, F[1], cfg.SH], dt.bfloat16,
                            addr_space="Shared"),
             nc.dram_tensor("h3full", [cfg.NCORE, F[2], cfg.SH], dt.bfloat16,
                            addr_space="Shared")]
    out = nc.dram_tensor("h4T", [F[3], cfg.SH], dt.float32,
                         kind="ExternalOutput")
    replica_groups = [list(range(cfg.NCORE))]

    with tile.TileContext(nc) as tc:
        with (
            tc.tile_pool(name="const", bufs=1) as cpool,
            tc.tile_pool(name="work", bufs=3) as pool,
            tc.tile_pool(name="big", bufs=3) as bigpool,
            tc.tile_pool(name="psum", bufs=2, space="PSUM") as psum,
        ):
            ident = cpool.tile([128, 128], dt.float32)
            make_identity(nc, ident[:])
            Rt, WAh_t, WAa_t, ba_t, PW_t = [], [], [], [], {}
            for l in range(3):
                t = cpool.tile([65, F[l + 1]], dt.bfloat16, tag=f"R{l}")
                nc.sync.dma_start(out=t[:], in_=inp[f"R{l}"][:, :])
                Rt.append(t)
                t = cpool.tile([F[l], F[l + 1]], dt.bfloat16, tag=f"WAh{l}")
                nc.sync.dma_start(out=t[:], in_=inp[f"WAh{l}"][:, :])
                WAh_t.append(t)
                t = cpool.tile([F[l + 1], F[l + 1]], dt.bfloat16, tag=f"WAa{l}")
                nc.sync.dma_start(out=t[:], in_=inp[f"WAa{l}"][:, :])
                WAa_t.append(t)
                t = cpool.tile([F[l + 1], 1], dt.float32, tag=f"ba{l}")
                nc.sync.dma_start(out=t[:], in_=inp[f"ba{l}"][:, :])
                ba_t.append(t)
            for l in (1, 2):
                t = cpool.tile([F[l], 64], dt.bfloat16, tag=f"PW{l}")
                nc.sync.dma_start(out=t[:], in_=inp[f"PW{l}"][:, :])
                PW_t[l] = t

            # zero agg tables
            ZB = min(cfg.NT, 14)
            zeros = cpool.tile([128, ZB * 64], dt.float32)
            nc.gpsimd.memset(zeros[:], 0.0)
            zv = zeros[:].rearrange("p (n m) -> p n m", m=64)
            for l in range(3):
                for t in agg[l]:
                    tv = t.rearrange("(n p) m -> p n m", p=128)
                    for t0 in range(0, cfg.NT, ZB):
                        w = min(ZB, cfg.NT - t0)
                        nc.sync.dma_start(out=tv[:, t0:t0 + w, :],
                                          in_=zv[:, :w, :])

            for l in range(3):
                Fm = F[l + 1]
                Fin = F[l]

                # ---- edge phase ----
                for c in range(cfg.NCHUNK):
                    b = c // cfg.CPB
                    gi = pool.tile([128, cfg.CG // 16], dt.int16, tag="gi")
                    nc.sync.dma_start(out=gi[:], in_=gidx[c, :, :])
                    si = pool.tile([128, cfg.CG // 16], dt.int16, tag="si")
                    nc.sync.dma_start(out=si[:], in_=sidx[c, :, :])
                    pg = bigpool.tile([128, cfg.NG, 64], dt.float32, tag="pg")
                    gn = cfg.CG if "gather" not in ablate else 128
                    nc.gpsimd.dma_gather(
                        out_ap=pg[:, :gn // 128, :],
                        in_ap=ptab[l][b * cfg.BUCK:(b + 1) * cfg.BUCK, :],
                        idxs_ap=gi[:, :max(gn // 16, 8)],
                        num_idxs=gn,
                        num_idxs_reg=gn,
                        elem_size=64,
                        queue_num=0,
                        single_packet=False,
                    )
                    et = bigpool.tile([65, cfg.CG], dt.bfloat16, tag="et")
                    nc.sync.dma_start(
                        out=et[:], in_=eT[:, c * cfg.CG:(c + 1) * cfg.CG])
                    m = bigpool.tile([128, cfg.NG, Fm], dt.float32, tag="m")
                    for g0 in range(0, cfg.NG, cfg.PSG):
                        qp = psum.tile([128, cfg.PSG, Fm], dt.float32,
                                       tag="qp")
                        for g in range(g0, g0 + (cfg.PSG if "mm" not in ablate else 1)):
                            nc.tensor.matmul(
                                qp[:, g - g0, :],
                                lhsT=et[:, g * 128:(g + 1) * 128],
                                rhs=Rt[l][:],
                                start=True, stop=True,
                            )
                        nc.vector.tensor_tensor(
                            out=m[:, g0:g0 + cfg.PSG, :],
                            in0=qp[:],
                            in1=pg[:, g0:g0 + cfg.PSG, :Fm],
                            op=mybir.AluOpType.add,
                        )
                        nc.scalar.activation(
                            m[:, g0:g0 + cfg.PSG, :],
                            m[:, g0:g0 + cfg.PSG, :],
                            mybir.ActivationFunctionType.Relu,
                        )
                    nhalf = cfg.CS // 128
                    sn = cfg.CS if "scatter" not in ablate else 128
                    for h in range(2):
                        nc.gpsimd.dma_scatter_add(
                            out_ap=agg[l][h][:, :Fm],
                            in_ap=m[:, h * nhalf:h * nhalf + sn // 128, :],
                            idxs_ap=si[:, h * (cfg.CS // 16):
                                       h * (cfg.CS // 16) + max(sn // 16, 8)],
                            num_idxs=sn,
                            num_idxs_reg=sn,
                            elem_size=Fm,
                            elem_step=64,
                            queue_num=0,
                            single_packet=False,
                        )

                # ---- node apply ----
                AB = min(cfg.NT, 14)
                aggT = cpool.tile([64, cfg.SH], dt.bfloat16, tag="aggT")
                agv0 = agg[l][0].rearrange("(n p) m -> p n m", p=128)
                agv1 = agg[l][1].rearrange("(n p) m -> p n m", p=128)
                for t0 in range(0, cfg.NT, AB):
                    w = min(AB, cfg.NT - t0)
                    ap_ = pool.tile([128, AB, 64], dt.float32, tag="aggp")
                    bp_ = pool.tile([128, AB, 64], dt.float32, tag="aggq")
                    nc.sync.dma_start(out=ap_[:, :w, :],
                                      in_=agv0[:, t0:t0 + w, :])
                    nc.sync.dma_start(out=bp_[:, :w, :],
                                      in_=agv1[:, t0:t0 + w, :])
                    nc.vector.tensor_tensor(out=ap_[:, :w, :],
                                            in0=ap_[:, :w, :],
                                            in1=bp_[:, :w, :],
                                            op=mybir.AluOpType.add)
                    for t in range(w):
                        tp = psum.tile([64, 128], dt.float32, tag="tp")
                        nc.tensor.transpose(out=tp[:], in_=ap_[:, t, :],
                                            identity=ident[:])
                        nc.vector.tensor_copy(
                            out=aggT[:, (t0 + t) * 128:(t0 + t + 1) * 128],
                            in_=tp[:])
                rhs1 = cpool.tile([64, cfg.SH], dt.bfloat16, tag="rhs")
                nc.sync.dma_start(out=rhs1[:Fin, :], in_=hsh[l][:, :])
                odt = dt.bfloat16 if l < 2 else dt.float32
                for s in range(0, cfg.SH, 512):
                    w = min(512, cfg.SH - s)
                    npz = psum.tile([Fm, 512], dt.float32, tag="np")
                    nc.tensor.matmul(npz[:, :w], lhsT=WAh_t[l][:],
                                     rhs=rhs1[:Fin, s:s + w],
                                     start=True, stop=False)
                    nc.tensor.matmul(npz[:, :w], lhsT=WAa_t[l][:],
                                     rhs=aggT[:Fm, s:s + w],
                                     start=False, stop=True)
                    hsl = pool.tile([Fm, 512], odt, tag="hsl")
                    nc.scalar.activation(hsl[:, :w], npz[:, :w],
                                         mybir.ActivationFunctionType.Relu,
                                         bias=ba_t[l][:])
                    tgt = hsh[l + 1] if l < 2 else out
                    nc.sync.dma_start(out=tgt[:, s:s + w], in_=hsl[:, :w])

                if l < 2:
                    if collectives:
                        nc.gpsimd.collective_compute(
                            "AllGather",
                            mybir.AluOpType.bypass,
                            ins=[hsh[l + 1][:, :]],
                            outs=[hfull[l + 1][:, :, :]],
                            replica_groups=replica_groups,
                        )
                    else:
                        for s8 in range(cfg.NCORE):
                            nc.sync.dma_start(out=hfull[l + 1][s8, :, :],
                                              in_=hsh[l + 1][:, :])
                    Fn = F[l + 1]
                    PB = min(cfg.NT, 14)
                    ptv = ptab[l + 1].rearrange("(n p) m -> p n m", p=128)
                    for s8 in range(cfg.NCORE):
                        for t0 in range(0, cfg.NT, PB):
                            wt = min(PB, cfg.NT - t0)
                            hs = pool.tile([Fn, PB * 128], dt.bfloat16,
                                           tag="hs")
                            nc.sync.dma_start(
                                out=hs[:, :wt * 128],
                                in_=hfull[l + 1][s8, :,
                                                 t0 * 128:(t0 + wt) * 128])
                            pt = pool.tile([128, PB, 64], dt.float32,
                                           tag="pt")
                            for t in range(t0, t0 + wt):
                                pp = psum.tile([128, 64], dt.float32,
                                               tag="pp")
                                nc.tensor.matmul(
                                    pp[:],
                                    lhsT=hs[:, (t - t0) * 128:
                                            (t - t0 + 1) * 128],
                                    rhs=PW_t[l + 1][:],
                                    start=True, stop=True)
                                nc.vector.tensor_copy(out=pt[:, t - t0, :],
                                                      in_=pp[:])
                            nc.sync.dma_start(
                                out=ptv[:, s8 * cfg.NT + t0:
                                        s8 * cfg.NT + t0 + wt, :],
                                in_=pt[:, :wt, :])

    nc.compile()
    return nc


# --------------------------------------------------------------------------
# Host-side preprocessing
# --------------------------------------------------------------------------

def host_prep(cfg, nfeats, efeats, src, dst, W):
    """Build per-core input maps.

    Edge layout per core: 4 src-buckets x K scatter bins of CS positions.
    Within a (core, bucket), each destination's edges are dealt round-robin
    across the K bins so every bin is duplicate-free per destination
    (required by dma_scatter_add).  Within a bin, edges are src-sorted for
    gather locality.  Pad slots gather row 0 and scatter into the trash row.
    """
    F = cfg.F
    n = cfg.N
    src = np.asarray(src).astype(np.int64).reshape(-1)
    dst = np.asarray(dst).astype(np.int64).reshape(-1)
    nfeats = np.asarray(nfeats, np.float32).reshape(n, 64)
    efeats = np.asarray(efeats, np.float32).reshape(-1, 64)

    nid = np.arange(n)
    prow_of_node = (nid // cfg.NSH) * cfg.SH + nid % cfg.NSH
    prow = prow_of_node[src]
    core = dst // cfg.NSH

    shared = {}
    p1 = nfeats @ np.asarray(W["Wm1"], np.float32)[:64]
    p1tab = np.zeros((cfg.NTAB, 64), np.float32)
    p1tab[prow_of_node, :F[1]] = p1
    shared["p1tab"] = p1tab
    for l, (wm, bm, wa, ba) in enumerate(
            [(W["Wm1"], W["bm1"], W["Wa1"], W["ba1"]),
             (W["Wm2"], W["bm2"], W["Wa2"], W["ba2"]),
             (W["Wm3"], W["bm3"], W["Wa3"], W["ba3"])]):
        wm = np.asarray(wm, np.float32)
        wa = np.asarray(wa, np.float32)
        fin, fm = F[l], F[l + 1]
        assert wm.shape == (fin + 64, fm) and wa.shape == (fin + fm, fm)
        shared[f"R{l}"] = np.concatenate(
            [wm[fin:], np.asarray(bm, np.float32)[None]], 0).astype(BF16)
        shared[f"WAh{l}"] = wa[:fin].astype(BF16)
        shared[f"WAa{l}"] = wa[fin:].astype(BF16)
        shared[f"ba{l}"] = np.asarray(ba, np.float32).reshape(fm, 1)
        if l >= 1:
            pw = np.zeros((fin, 64), BF16)
            pw[:, :fm] = wm[:fin].astype(BF16)
            shared[f"PW{l}"] = pw

    def wrap(idx16, width):
        """[NCHUNK*width] -> [NCHUNK, 128, width//16] wrapped+replicated."""
        w = idx16.reshape(-1, width // 16, 16).transpose(0, 2, 1)
        return np.ascontiguousarray(
            np.broadcast_to(w[:, None], (w.shape[0], 8, 16, w.shape[2]))
            .reshape(w.shape[0], 128, w.shape[2]))

    in_maps = []
    for k in range(cfg.NCORE):
        ids = np.nonzero(core == k)[0]
        gq = np.zeros(cfg.EPAD, np.int16)
        sq = np.full(cfg.EPAD, cfg.TRASH, np.int16)
        epos = np.empty(cfg.EPAD, np.int64)   # edge id per slot (-1 = pad)
        evalid = np.zeros(cfg.EPAD, bool)
        bucket_of = prow[ids] // cfg.BUCK
        for b in range(cfg.NBUCK):
            sel = ids[bucket_of == b]
            # sort by dst so each dst's edges are consecutive, then deal
            # edge j of dst across bins (start_dst + j) % K
            o = np.argsort(dst[sel], kind="stable")
            sel = sel[o]
            d = dst[sel]
            cnt = sel.shape[0]
            assert cnt <= cfg.CAP, (k, b, cnt)
            # dst-sorted + cyclic deal: run of length <=K -> distinct bins
            starts = np.nonzero(np.diff(d, prepend=-1))[0]
            run_lens = np.diff(np.append(starts, cnt))
            assert cnt == 0 or int(run_lens.max()) <= cfg.K, (k, b)
            bins = np.arange(cnt) % cfg.K
            # place: within each bin, src-sorted
            o2 = np.lexsort((prow[sel], bins))
            sel = sel[o2]
            bins = bins[o2]
            fill = np.bincount(bins, minlength=cfg.K)
            assert fill.max() <= cfg.CS, (k, b, fill.max())
            off_in_bin = np.arange(cnt) - np.repeat(
                np.cumsum(np.append(0, fill))[:-1], fill)
            slot = b * cfg.CAP + bins * cfg.CS + off_in_bin
            gq[slot] = (prow[sel] - b * cfg.BUCK).astype(np.int16)
            sq[slot] = (dst[sel] - k * cfg.NSH).astype(np.int16)
            epos[slot] = sel
            evalid[slot] = True
        eT = np.zeros((65, cfg.EPAD), BF16)
        eT[64] = 1.0
        vs = np.nonzero(evalid)[0]
        eT[:64, vs] = efeats[epos[vs]].T.astype(BF16)
        hT1own = np.zeros((64, cfg.SH), BF16)
        hT1own[:, :cfg.NSH] = \
            nfeats[k * cfg.NSH:(k + 1) * cfg.NSH].T.astype(BF16)
        im = dict(shared)
        im["eT"] = eT
        im["gidx"] = wrap(gq, cfg.CG)
        im["sidx"] = wrap(sq, cfg.CG)
        im["hT1own"] = hT1own
        in_maps.append(im)
    return in_maps


def reference_small(cfg, nfeats, efeats, src, dst, W):
    h = np.asarray(nfeats, np.float32).reshape(cfg.N, 64)
    e = np.asarray(efeats, np.float32).reshape(-1, 64)
    src = np.asarray(src).reshape(-1)
    dst = np.asarray(dst).reshape(-1)
    for (wm, bm, wa, ba) in [(W["Wm1"], W["bm1"], W["Wa1"], W["ba1"]),
                             (W["Wm2"], W["bm2"], W["Wa2"], W["ba2"]),
                             (W["Wm3"], W["bm3"], W["Wa3"], W["ba3"])]:
        m = np.maximum(
            np.concatenate([h[src], e], 1) @ np.asarray(wm, np.float32)
            + np.asarray(bm, np.float32), 0)
        agg = np.zeros((cfg.N, m.shape[1]), np.float32)
        np.add.at(agg, dst, m)
        h = np.maximum(
            np.concatenate([h, agg], 1) @ np.asarray(wa, np.float32)
            + np.asarray(ba, np.float32), 0)
    return h


# --------------------------------------------------------------------------
# Entry point
# --------------------------------------------------------------------------

LAST_RESULTS = None
LAST_EXEC_NS = None

_NC_CACHE = {}


def _get_nc(cfg):
    if id(cfg) not in _NC_CACHE:
        _NC_CACHE[id(cfg)] = build_nc(cfg)
    return _NC_CACHE[id(cfg)]


def _bench_exec(nc, in_maps, iters=3):
    """Time warm executions of the compiled NEFF with device-resident
    inputs (no NTFF profiling available under this axon setup).  Returns
    best wall seconds for one 8-core dispatch+execute."""
    import time
    import jax
    import numpy as np_
    from jax.sharding import Mesh, PartitionSpec, NamedSharding
    from concourse import bass2jax, mybir

    n_cores = len(in_maps)
    partition_name = (nc.partition_id_tensor.name
                      if nc.partition_id_tensor else None)
    in_names, out_names, out_avals, zero_outs = [], [], [], []
    for alloc in nc.m.functions[0].allocations:
        if not isinstance(alloc, mybir.MemoryLocationSet):
            continue
        name = alloc.memorylocations[0].name
        if alloc.kind == "ExternalInput":
            if name != partition_name:
                in_names.append(name)
        elif alloc.kind == "ExternalOutput":
            shape = tuple(alloc.tensor_shape)
            dtype = mybir.dt.np(alloc.dtype)
            out_names.append(name)
            out_avals.append(jax.core.ShapedArray(shape, dtype))
            zero_outs.append(np_.zeros((n_cores * shape[0], *shape[1:]),
                                       dtype))
    n_params = len(in_names)
    all_in_names = in_names + out_names + (
        [partition_name] if partition_name else [])

    def _body(*args):
        operands = list(args)
        if partition_name is not None:
            operands.append(bass2jax.partition_id_tensor())
        return tuple(bass2jax._bass_exec_p.bind(
            *operands,
            out_avals=tuple(out_avals),
            in_names=tuple(all_in_names),
            out_names=tuple(out_names),
            lowering_input_output_aliases=(),
            sim_require_finite=True,
            sim_require_nnan=True,
            nc=nc,
        ))

    devices = jax.devices()[:n_cores]
    mesh = Mesh(np_.asarray(devices), ("core",))
    donate = tuple(range(n_params, n_params + len(out_names)))
    sharded = jax.jit(
        bass2jax.shard_map(_body, mesh=mesh,
                           in_specs=(PartitionSpec("core"),) * (
                               n_params + len(out_names)),
                           out_specs=(PartitionSpec("core"),) * len(out_names),
                           check_rep=False),
        donate_argnums=donate, keep_unused=True)
    sh = NamedSharding(mesh, PartitionSpec("core"))
    dev_in = [jax.device_put(
        np_.concatenate([np_.asarray(m[n]) for m in in_maps], axis=0), sh)
        for n in in_names]
    best = None
    for _ in range(iters):
        zs = [np_.zeros_like(z) for z in zero_outs]
        t0 = time.perf_counter()
        outs = sharded(*dev_in, *zs)
        jax.block_until_ready(outs)
        dt_ = time.perf_counter() - t0
        best = dt_ if best is None else min(best, dt_)
    return best


def kernel(nfeats, efeats, src, dst,
           Wm1, bm1, Wa1, ba1,
           Wm2, bm2, Wa2, ba2,
           Wm3, bm3, Wa3, ba3):
    import os
    from concourse.bass_utils import run_bass_kernel_spmd

    cfg = FULL
    W = dict(Wm1=Wm1, bm1=bm1, Wa1=Wa1, ba1=ba1,
             Wm2=Wm2, bm2=bm2, Wa2=Wa2, ba2=ba2,
             Wm3=Wm3, bm3=bm3, Wa3=Wa3, ba3=ba3)
    in_maps = host_prep(cfg, nfeats, efeats, src, dst, W)
    nc = _get_nc(cfg)
    res = run_bass_kernel_spmd(nc, in_maps, core_ids=list(range(cfg.NCORE)))
    global LAST_RESULTS, LAST_EXEC_NS
    LAST_RESULTS = res
    if os.environ.get("GNN_BENCH"):
        LAST_EXEC_NS = int(_bench_exec(nc, in_maps) * 1e9)
    out = np.empty((cfg.N, 32), np.float32)
    for k in range(cfg.NCORE):
        out[k * cfg.NSH:(k + 1) * cfg.NSH] = \
            np.asarray(res.results[k]["h4T"])[:, :cfg.NSH].T
    return out


# revision 18
# speedup vs baseline: 1.0573x; 1.0573x over previous
"""GNN message-passing kernel for nn_GCN2_64630667870322 on 8 Trainium2 cores.

3-layer edge-message GNN:
    m_e   = relu(cat(h[src_e], efeat_e) @ Wm + bm)        per edge
    agg_v = sum_{e: dst_e == v} m_e                        segment sum
    h'_v  = relu(cat(h_v, agg_v) @ Wa + ba)                per node
Output: h3 squeezed -> [N, 32] f32.

Strategy (edge-parallel, dst-range sharded across the 8 cores):
  * Core k owns dst nodes [k*12500, (k+1)*12500); its edges are exactly
    those with dst in range, so per-destination partial sums stay local.
  * Message decomposition: m = relu(p[src] + q),
      p = h @ Wm[:Fin]            (node-level matmul, f32 table)
      q = cat(e, 1) @ [Wm[Fin:]; bm]   (edge matmul on PE, bias folded)
  * p[src]: dma_gather (custom SWDGE op) of 256B rows; node ids bucketed
    into 4 windows of <=32767 rows so indices fit int16.
  * Aggregation: dma_scatter_add into per-core HBM tables.  The HW op
    loses updates for duplicate indices WITHIN one instruction (verified
    on silicon), but separate instructions accumulate exactly.  Host-side
    bin packing therefore deals each destination's edges across bins of
    2048 positions such that every bin is duplicate-free; each bin is one
    scatter instruction.  Bins alternate between SWDGE queues 1/2 and two
    agg tables (summed during node apply) for descriptor-gen parallelism.
  * Between layers, per-core node states h'^T (feature-major, bf16) are
    AllGather'd; every core rebuilds the full f32 p-table locally.

kernel(**inputs) accepts FULL inputs, returns the FULL [100000, 32] f32
output; all sharding happens inside.
"""
import numpy as np
import ml_dtypes

BF16 = ml_dtypes.bfloat16


class Cfg:
    def __init__(self, n_nodes, bins_per_bucket, cg=4096, cs=2048, psg=8):
        self.NCORE = 8
        self.NBUCK = 4
        self.N = n_nodes
        assert n_nodes % self.NCORE == 0
        self.NSH = n_nodes // self.NCORE               # real nodes per core
        self.SH = ((self.NSH + 127) // 128 + 1) * 128  # padded rows (+trash)
        self.NTAB = self.NCORE * self.SH               # p-table rows
        assert self.NTAB % self.NBUCK == 0
        self.BUCK = self.NTAB // self.NBUCK            # gather window rows
        assert self.BUCK <= 32767 and self.SH <= 32767
        assert self.BUCK % self.SH == 0
        self.CG = cg                                   # gather/edge chunk
        self.CS = cs                                   # scatter bin size
        assert cg % cs == 0 and cs % 128 == 0
        self.K = bins_per_bucket                       # scatter bins/bucket
        assert (self.K * cs) % cg == 0
        self.CAP = self.K * cs                         # edges per (core,bucket)
        self.CPB = self.CAP // cg                      # chunks per bucket
        self.NCHUNK = self.NBUCK * self.CPB
        self.EPAD = self.NCHUNK * cg
        self.TRASH = self.SH - 1
        self.F = [64, 50, 25, 32]
        self.NG = cg // 128                            # edge groups / chunk
        self.PSG = psg                                 # groups per psum tile
        assert self.NG % self.PSG == 0
        self.NT = self.SH // 128                       # 128-row tiles / shard


FULL = Cfg(100000, bins_per_bucket=52, cg=8192, cs=2048, psg=16)


# --------------------------------------------------------------------------
# Device program (SPMD — identical on all 8 cores)
# --------------------------------------------------------------------------

def build_nc(cfg, collectives=True, ablate=()):
    from concourse import bacc, mybir, tile
    from concourse.masks import make_identity

    dt = mybir.dt
    nc = bacc.Bacc(
        "TRN2",
        target_bir_lowering=False,
        debug=False,
        dynamic_dma_scratch_size=32768,
    )
    F = cfg.F

    inp = {}

    def ext(name, shape, dtype):
        inp[name] = nc.dram_tensor(name, shape, dtype, kind="ExternalInput")
        return inp[name]

    p1tab = ext("p1tab", [cfg.NTAB, 64], dt.float32)
    eT = ext("eT", [65, cfg.EPAD], dt.bfloat16)
    gidx = ext("gidx", [cfg.NCHUNK, 128, cfg.CG // 16], dt.int16)
    sidx = ext("sidx", [cfg.NCHUNK, 128, cfg.CG // 16], dt.int16)
    hT1own = ext("hT1own", [64, cfg.SH], dt.bfloat16)
    for l in range(3):
        ext(f"R{l}", [65, F[l + 1]], dt.bfloat16)
        ext(f"WAh{l}", [F[l], F[l + 1]], dt.bfloat16)
        ext(f"WAa{l}", [F[l + 1], F[l + 1]], dt.bfloat16)
        ext(f"ba{l}", [F[l + 1], 1], dt.float32)
    for l in (1, 2):
        ext(f"PW{l}", [F[l], 64], dt.bfloat16)

    agg = [[nc.dram_tensor(f"agg{l}{ab}", [cfg.SH, 64], dt.float32)
            for ab in "ab"] for l in range(3)]
    ptab = [p1tab,
            nc.dram_tensor("p2tab", [cfg.NTAB, 64], dt.float32),
            nc.dram_tensor("p3tab", [cfg.NTAB, 64], dt.float32)]
    hsh = [hT1own,
           nc.dram_tensor("h2sh", [F[1], cfg.SH], dt.bfloat16),
           nc.dram_tensor("h3sh", [F[2], cfg.SH], dt.bfloat16)]
    hfull = [None,
             nc.dram_tensor("h2full", [cfg.NCORE, F[1], cfg.SH], dt.bfloat16,
                            addr_space="Shared"),
             nc.dram_tensor("h3full", [cfg.NCORE, F[2], cfg.SH], dt.bfloat16,
                            addr_space="Shared")]
    out = nc.dram_tensor("h4T", [F[3], cfg.SH], dt.float32,
                         kind="ExternalOutput")
    replica_groups = [list(range(cfg.NCORE))]

    with tile.TileContext(nc) as tc:
        with (
            tc.tile_pool(name="const", bufs=1) as cpool,
            tc.tile_pool(name="work", bufs=3) as pool,
            tc.tile_pool(name="big", bufs=2) as bigpool,
            tc.tile_pool(name="psum", bufs=2, space="PSUM") as psum,
            tc.tile_pool(name="psum1", bufs=1, space="PSUM") as psum1,
        ):
            ident = cpool.tile([128, 128], dt.float32)
            make_identity(nc, ident[:])
            Rt, WAh_t, WAa_t, ba_t, PW_t = [], [], [], [], {}
            for l in range(3):
                t = cpool.tile([65, F[l + 1]], dt.bfloat16, tag=f"R{l}")
                nc.sync.dma_start(out=t[:], in_=inp[f"R{l}"][:, :])
                Rt.append(t)
                t = cpool.tile([F[l], F[l + 1]], dt.bfloat16, tag=f"WAh{l}")
                nc.sync.dma_start(out=t[:], in_=inp[f"WAh{l}"][:, :])
                WAh_t.append(t)
                t = cpool.tile([F[l + 1], F[l + 1]], dt.bfloat16, tag=f"WAa{l}")
                nc.sync.dma_start(out=t[:], in_=inp[f"WAa{l}"][:, :])
                WAa_t.append(t)
                t = cpool.tile([F[l + 1], 1], dt.float32, tag=f"ba{l}")
                nc.sync.dma_start(out=t[:], in_=inp[f"ba{l}"][:, :])
                ba_t.append(t)
            for l in (1, 2):
                t = cpool.tile([F[l], 64], dt.bfloat16, tag=f"PW{l}")
                nc.sync.dma_start(out=t[:], in_=inp[f"PW{l}"][:, :])
                PW_t[l] = t

            # zero agg tables
            ZB = min(cfg.NT, 14)
            zeros = cpool.tile([128, ZB * 64], dt.float32)
            nc.gpsimd.memset(zeros[:], 0.0)
            zv = zeros[:].rearrange("p (n m) -> p n m", m=64)
            for l in range(3):
                for t in agg[l]:
                    tv = t.rearrange("(n p) m -> p n m", p=128)
                    for t0 in range(0, cfg.NT, ZB):
                        w = min(ZB, cfg.NT - t0)
                        nc.sync.dma_start(out=tv[:, t0:t0 + w, :],
                                          in_=zv[:, :w, :])

            for l in range(3):
                Fm = F[l + 1]
                Fin = F[l]

                # ---- edge phase ----
                for c in range(cfg.NCHUNK):
                    b = c // cfg.CPB
                    gi = pool.tile([128, cfg.CG // 16], dt.int16, tag="gi")
                    nc.sync.dma_start(out=gi[:], in_=gidx[c, :, :])
                    si = pool.tile([128, cfg.CG // 16], dt.int16, tag="si")
                    nc.sync.dma_start(out=si[:], in_=sidx[c, :, :])
                    pg = bigpool.tile([128, cfg.NG, 64], dt.float32, tag="pg")
                    gn = cfg.CG if "gather" not in ablate else 128
                    nc.gpsimd.dma_gather(
                        out_ap=pg[:, :gn // 128, :],
                        in_ap=ptab[l][b * cfg.BUCK:(b + 1) * cfg.BUCK, :],
                        idxs_ap=gi[:, :max(gn // 16, 8)],
                        num_idxs=gn,
                        num_idxs_reg=gn,
                        elem_size=64,
                        queue_num=0,
                        single_packet=False,
                    )
                    m = bigpool.tile([128, cfg.NG, Fm], dt.float32, tag="m")
                    EH = cfg.CG // 2
                    et = None
                    for g0 in range(0, cfg.NG, cfg.PSG):
                        if g0 * 128 % EH == 0:
                            eh = (g0 * 128) // EH
                            et = bigpool.tile([65, EH], dt.bfloat16, tag="et")
                            nc.sync.dma_start(
                                out=et[:],
                                in_=eT[:, c * cfg.CG + eh * EH:
                                       c * cfg.CG + (eh + 1) * EH])
                        qp = psum.tile([128, cfg.PSG, Fm], dt.float32,
                                       tag="qp")
                        for g in range(g0, g0 + (cfg.PSG if "mm" not in ablate else 1)):
                            ge = g * 128 - (g0 * 128 // EH) * EH
                            nc.tensor.matmul(
                                qp[:, g - g0, :],
                                lhsT=et[:, ge:ge + 128],
                                rhs=Rt[l][:],
                                start=True, stop=True,
                            )
                        nc.vector.tensor_tensor(
                            out=m[:, g0:g0 + cfg.PSG, :],
                            in0=qp[:],
                            in1=pg[:, g0:g0 + cfg.PSG, :Fm],
                            op=mybir.AluOpType.add,
                        )
                        nc.scalar.activation(
                            m[:, g0:g0 + cfg.PSG, :],
                            m[:, g0:g0 + cfg.PSG, :],
                            mybir.ActivationFunctionType.Relu,
                        )
                    nhalf = cfg.CS // 128
                    sn = cfg.CS if "scatter" not in ablate else 128
                    for h in range(cfg.CG // cfg.CS):
                        nc.gpsimd.dma_scatter_add(
                            out_ap=agg[l][h % 2][:, :Fm],
                            in_ap=m[:, h * nhalf:h * nhalf + sn // 128, :],
                            idxs_ap=si[:, h * (cfg.CS // 16):
                                       h * (cfg.CS // 16) + max(sn // 16, 8)],
                            num_idxs=sn,
                            num_idxs_reg=sn,
                            elem_size=Fm,
                            elem_step=64,
                            queue_num=0,
                            single_packet=False,
                        )

                # ---- node apply ----
                AB = min(cfg.NT, 14)
                aggT = cpool.tile([64, cfg.SH], dt.bfloat16, tag="aggT")
                agv0 = agg[l][0].rearrange("(n p) m -> p n m", p=128)
                agv1 = agg[l][1].rearrange("(n p) m -> p n m", p=128)
                for t0 in range(0, cfg.NT, AB):
                    w = min(AB, cfg.NT - t0)
                    ap_ = pool.tile([128, AB, 64], dt.float32, tag="aggp")
                    bp_ = pool.tile([128, AB, 64], dt.float32, tag="aggq")
                    nc.sync.dma_start(out=ap_[:, :w, :],
                                      in_=agv0[:, t0:t0 + w, :])
                    nc.sync.dma_start(out=bp_[:, :w, :],
                                      in_=agv1[:, t0:t0 + w, :])
                    nc.vector.tensor_tensor(out=ap_[:, :w, :],
                                            in0=ap_[:, :w, :],
                                            in1=bp_[:, :w, :],
                                            op=mybir.AluOpType.add)
                    for t in range(w):
                        tp = psum1.tile([64, 128], dt.float32, tag="tp")
                        nc.tensor.transpose(out=tp[:], in_=ap_[:, t, :],
                                            identity=ident[:])
                        nc.vector.tensor_copy(
                            out=aggT[:, (t0 + t) * 128:(t0 + t + 1) * 128],
                            in_=tp[:])
                rhs1 = cpool.tile([64, cfg.SH], dt.bfloat16, tag="rhs")
                nc.sync.dma_start(out=rhs1[:Fin, :], in_=hsh[l][:, :])
                odt = dt.bfloat16 if l < 2 else dt.float32
                for s in range(0, cfg.SH, 512):
                    w = min(512, cfg.SH - s)
                    npz = psum.tile([Fm, 512], dt.float32, tag="np")
                    nc.tensor.matmul(npz[:, :w], lhsT=WAh_t[l][:],
                                     rhs=rhs1[:Fin, s:s + w],
                                     start=True, stop=False)
                    nc.tensor.matmul(npz[:, :w], lhsT=WAa_t[l][:],
                                     rhs=aggT[:Fm, s:s + w],
                                     start=False, stop=True)
                    hsl = pool.tile([Fm, 512], odt, tag="hsl")
                    nc.scalar.activation(hsl[:, :w], npz[:, :w],
                                         mybir.ActivationFunctionType.Relu,
                                         bias=ba_t[l][:])
                    tgt = hsh[l + 1] if l < 2 else out
                    nc.sync.dma_start(out=tgt[:, s:s + w], in_=hsl[:, :w])

                if l < 2:
                    if collectives:
                        nc.gpsimd.collective_compute(
                            "AllGather",
                            mybir.AluOpType.bypass,
                            ins=[hsh[l + 1][:, :]],
                            outs=[hfull[l + 1][:, :, :]],
                            replica_groups=replica_groups,
                        )
                    else:
                        for s8 in range(cfg.NCORE):
                            nc.sync.dma_start(out=hfull[l + 1][s8, :, :],
                                              in_=hsh[l + 1][:, :])
                    Fn = F[l + 1]
                    PB = min(cfg.NT, 14)
                    ptv = ptab[l + 1].rearrange("(n p) m -> p n m", p=128)
                    for s8 in range(cfg.NCORE):
                        for t0 in range(0, cfg.NT, PB):
                            wt = min(PB, cfg.NT - t0)
                            hs = pool.tile([Fn, PB * 128], dt.bfloat16,
                                           tag="hs")
                            nc.sync.dma_start(
                                out=hs[:, :wt * 128],
                                in_=hfull[l + 1][s8, :,
                                                 t0 * 128:(t0 + wt) * 128])
                            pt = pool.tile([128, PB, 64], dt.float32,
                                           tag="pt")
                            for t in range(t0, t0 + wt):
                                pp = psum1.tile([128, 64], dt.float32,
                                                tag="pp")
                                nc.tensor.matmul(
                                    pp[:],
                                    lhsT=hs[:, (t - t0) * 128:
                                            (t - t0 + 1) * 128],
                                    rhs=PW_t[l + 1][:],
                                    start=True, stop=True)
                                nc.vector.tensor_copy(out=pt[:, t - t0, :],
                                                      in_=pp[:])
                            nc.sync.dma_start(
                                out=ptv[:, s8 * cfg.NT + t0:
                                        s8 * cfg.NT + t0 + wt, :],
                                in_=pt[:, :wt, :])

    nc.compile()
    return nc


# --------------------------------------------------------------------------
# Host-side preprocessing
# --------------------------------------------------------------------------

def host_prep(cfg, nfeats, efeats, src, dst, W):
    """Build per-core input maps.

    Edge layout per core: 4 src-buckets x K scatter bins of CS positions.
    Within a (core, bucket), each destination's edges are dealt round-robin
    across the K bins so every bin is duplicate-free per destination
    (required by dma_scatter_add).  Within a bin, edges are src-sorted for
    gather locality.  Pad slots gather row 0 and scatter into the trash row.
    """
    F = cfg.F
    n = cfg.N
    src = np.asarray(src).astype(np.int64).reshape(-1)
    dst = np.asarray(dst).astype(np.int64).reshape(-1)
    nfeats = np.asarray(nfeats, np.float32).reshape(n, 64)
    efeats = np.asarray(efeats, np.float32).reshape(-1, 64)

    nid = np.arange(n)
    prow_of_node = (nid // cfg.NSH) * cfg.SH + nid % cfg.NSH
    prow = prow_of_node[src]
    core = dst // cfg.NSH

    shared = {}
    p1 = nfeats @ np.asarray(W["Wm1"], np.float32)[:64]
    p1tab = np.zeros((cfg.NTAB, 64), np.float32)
    p1tab[prow_of_node, :F[1]] = p1
    shared["p1tab"] = p1tab
    for l, (wm, bm, wa, ba) in enumerate(
            [(W["Wm1"], W["bm1"], W["Wa1"], W["ba1"]),
             (W["Wm2"], W["bm2"], W["Wa2"], W["ba2"]),
             (W["Wm3"], W["bm3"], W["Wa3"], W["ba3"])]):
        wm = np.asarray(wm, np.float32)
        wa = np.asarray(wa, np.float32)
        fin, fm = F[l], F[l + 1]
        assert wm.shape == (fin + 64, fm) and wa.shape == (fin + fm, fm)
        shared[f"R{l}"] = np.concatenate(
            [wm[fin:], np.asarray(bm, np.float32)[None]], 0).astype(BF16)
        shared[f"WAh{l}"] = wa[:fin].astype(BF16)
        shared[f"WAa{l}"] = wa[fin:].astype(BF16)
        shared[f"ba{l}"] = np.asarray(ba, np.float32).reshape(fm, 1)
        if l >= 1:
            pw = np.zeros((fin, 64), BF16)
            pw[:, :fm] = wm[:fin].astype(BF16)
            shared[f"PW{l}"] = pw

    def wrap(idx16, width):
        """[NCHUNK*width] -> [NCHUNK, 128, width//16] wrapped+replicated."""
        w = idx16.reshape(-1, width // 16, 16).transpose(0, 2, 1)
        return np.ascontiguousarray(
            np.broadcast_to(w[:, None], (w.shape[0], 8, 16, w.shape[2]))
            .reshape(w.shape[0], 128, w.shape[2]))

    in_maps = []
    for k in range(cfg.NCORE):
        ids = np.nonzero(core == k)[0]
        gq = np.zeros(cfg.EPAD, np.int16)
        sq = np.full(cfg.EPAD, cfg.TRASH, np.int16)
        epos = np.empty(cfg.EPAD, np.int64)   # edge id per slot (-1 = pad)
        evalid = np.zeros(cfg.EPAD, bool)
        bucket_of = prow[ids] // cfg.BUCK
        for b in range(cfg.NBUCK):
            sel = ids[bucket_of == b]
            # sort by dst so each dst's edges are consecutive, then deal
            # edge j of dst across bins (start_dst + j) % K
            o = np.argsort(dst[sel], kind="stable")
            sel = sel[o]
            d = dst[sel]
            cnt = sel.shape[0]
            assert cnt <= cfg.CAP, (k, b, cnt)
            # dst-sorted + cyclic deal: run of length <=K -> distinct bins
            starts = np.nonzero(np.diff(d, prepend=-1))[0]
            run_lens = np.diff(np.append(starts, cnt))
            assert cnt == 0 or int(run_lens.max()) <= cfg.K, (k, b)
            bins = np.arange(cnt) % cfg.K
            # place: within each bin, src-sorted
            o2 = np.lexsort((prow[sel], bins))
            sel = sel[o2]
            bins = bins[o2]
            fill = np.bincount(bins, minlength=cfg.K)
            assert fill.max() <= cfg.CS, (k, b, fill.max())
            off_in_bin = np.arange(cnt) - np.repeat(
                np.cumsum(np.append(0, fill))[:-1], fill)
            slot = b * cfg.CAP + bins * cfg.CS + off_in_bin
            gq[slot] = (prow[sel] - b * cfg.BUCK).astype(np.int16)
            sq[slot] = (dst[sel] - k * cfg.NSH).astype(np.int16)
            epos[slot] = sel
            evalid[slot] = True
        eT = np.zeros((65, cfg.EPAD), BF16)
        eT[64] = 1.0
        vs = np.nonzero(evalid)[0]
        eT[:64, vs] = efeats[epos[vs]].T.astype(BF16)
        hT1own = np.zeros((64, cfg.SH), BF16)
        hT1own[:, :cfg.NSH] = \
            nfeats[k * cfg.NSH:(k + 1) * cfg.NSH].T.astype(BF16)
        im = dict(shared)
        im["eT"] = eT
        im["gidx"] = wrap(gq, cfg.CG)
        im["sidx"] = wrap(sq, cfg.CG)
        im["hT1own"] = hT1own
        in_maps.append(im)
    return in_maps


def reference_small(cfg, nfeats, efeats, src, dst, W):
    h = np.asarray(nfeats, np.float32).reshape(cfg.N, 64)
    e = np.asarray(efeats, np.float32).reshape(-1, 64)
    src = np.asarray(src).reshape(-1)
    dst = np.asarray(dst).reshape(-1)
    for (wm, bm, wa, ba) in [(W["Wm1"], W["bm1"], W["Wa1"], W["ba1"]),
                             (W["Wm2"], W["bm2"], W["Wa2"], W["ba2"]),
                             (W["Wm3"], W["bm3"], W["Wa3"], W["ba3"])]:
        m = np.maximum(
            np.concatenate([h[src], e], 1) @ np.asarray(wm, np.float32)
            + np.asarray(bm, np.float32), 0)
        agg = np.zeros((cfg.N, m.shape[1]), np.float32)
        np.add.at(agg, dst, m)
        h = np.maximum(
            np.concatenate([h, agg], 1) @ np.asarray(wa, np.float32)
            + np.asarray(ba, np.float32), 0)
    return h


# --------------------------------------------------------------------------
# Entry point
# --------------------------------------------------------------------------

LAST_RESULTS = None
LAST_EXEC_NS = None

_NC_CACHE = {}


def _get_nc(cfg):
    if id(cfg) not in _NC_CACHE:
        _NC_CACHE[id(cfg)] = build_nc(cfg)
    return _NC_CACHE[id(cfg)]


def _bench_exec(nc, in_maps, iters=3):
    """Time warm executions of the compiled NEFF with device-resident
    inputs (no NTFF profiling available under this axon setup).  Returns
    best wall seconds for one 8-core dispatch+execute."""
    import time
    import jax
    import numpy as np_
    from jax.sharding import Mesh, PartitionSpec, NamedSharding
    from concourse import bass2jax, mybir

    n_cores = len(in_maps)
    partition_name = (nc.partition_id_tensor.name
                      if nc.partition_id_tensor else None)
    in_names, out_names, out_avals, zero_outs = [], [], [], []
    for alloc in nc.m.functions[0].allocations:
        if not isinstance(alloc, mybir.MemoryLocationSet):
            continue
        name = alloc.memorylocations[0].name
        if alloc.kind == "ExternalInput":
            if name != partition_name:
                in_names.append(name)
        elif alloc.kind == "ExternalOutput":
            shape = tuple(alloc.tensor_shape)
            dtype = mybir.dt.np(alloc.dtype)
            out_names.append(name)
            out_avals.append(jax.core.ShapedArray(shape, dtype))
            zero_outs.append(np_.zeros((n_cores * shape[0], *shape[1:]),
                                       dtype))
    n_params = len(in_names)
    all_in_names = in_names + out_names + (
        [partition_name] if partition_name else [])

    def _body(*args):
        operands = list(args)
        if partition_name is not None:
            operands.append(bass2jax.partition_id_tensor())
        return tuple(bass2jax._bass_exec_p.bind(
            *operands,
            out_avals=tuple(out_avals),
            in_names=tuple(all_in_names),
            out_names=tuple(out_names),
            lowering_input_output_aliases=(),
            sim_require_finite=True,
            sim_require_nnan=True,
            nc=nc,
        ))

    devices = jax.devices()[:n_cores]
    mesh = Mesh(np_.asarray(devices), ("core",))
    donate = tuple(range(n_params, n_params + len(out_names)))
    sharded = jax.jit(
        bass2jax.shard_map(_body, mesh=mesh,
                           in_specs=(PartitionSpec("core"),) * (
                               n_params + len(out_names)),
                           out_specs=(PartitionSpec("core"),) * len(out_names),
                           check_rep=False),
        donate_argnums=donate, keep_unused=True)
    sh = NamedSharding(mesh, PartitionSpec("core"))
    dev_in = [jax.device_put(
        np_.concatenate([np_.asarray(m[n]) for m in in_maps], axis=0), sh)
        for n in in_names]
    best = None
    for _ in range(iters):
        zs = [np_.zeros_like(z) for z in zero_outs]
        t0 = time.perf_counter()
        outs = sharded(*dev_in, *zs)
        jax.block_until_ready(outs)
        dt_ = time.perf_counter() - t0
        best = dt_ if best is None else min(best, dt_)
    return best


def kernel(nfeats, efeats, src, dst,
           Wm1, bm1, Wa1, ba1,
           Wm2, bm2, Wa2, ba2,
           Wm3, bm3, Wa3, ba3):
    import os
    from concourse.bass_utils import run_bass_kernel_spmd

    cfg = FULL
    W = dict(Wm1=Wm1, bm1=bm1, Wa1=Wa1, ba1=ba1,
             Wm2=Wm2, bm2=bm2, Wa2=Wa2, ba2=ba2,
             Wm3=Wm3, bm3=bm3, Wa3=Wa3, ba3=ba3)
    in_maps = host_prep(cfg, nfeats, efeats, src, dst, W)
    nc = _get_nc(cfg)
    res = run_bass_kernel_spmd(nc, in_maps, core_ids=list(range(cfg.NCORE)))
    global LAST_RESULTS, LAST_EXEC_NS
    LAST_RESULTS = res
    if os.environ.get("GNN_BENCH"):
        LAST_EXEC_NS = int(_bench_exec(nc, in_maps) * 1e9)
    out = np.empty((cfg.N, 32), np.float32)
    for k in range(cfg.NCORE):
        out[k * cfg.NSH:(k + 1) * cfg.NSH] = \
            np.asarray(res.results[k]["h4T"])[:, :cfg.NSH].T
    return out
